# revision 8
# baseline (speedup 1.0000x reference)
"""DSDM memory-update kernel for Trainium2, SPMD across 8 NeuronCores.

Computation (per reference):
    d_i   = ||A_i - q_a||_2                      (i over 2M rows)
    min_d = min_i d_i
    new_ge = ge + ema_temp*(min_d - ge)
    append = min_d >= 0.95*new_ge
    w_i   = softmax(-d)_i * ema   (zeroed when append)
    A'    = A + w*(q_a - A);  M' = M + w*(q_c - M)
    out   = concat([A', M'], axis=1)

v2 strategy (memory-regime): the kernel is HBM-bound (A read twice + M read +
OUT written = 212 MB/core at ~358 GB/s). Cut traffic by loading A as bf16 via
SWDGE cast-DMA and keeping `n_cache` of the 31 row-tiles resident in SBUF
between the distance pass and the update pass; only the rest are re-read.
All heavy elementwise runs in bf16 (2x DVE mode), the per-row scales are
replicated across columns on the idle PE (outer products w^T@qblk and
s^T@onesblk), and OUT is staged bf16 in SBUF and cast-written to f32 by the
DMA. Verified numerically: bf16 arithmetic gives max rel err ~0.6% vs the
f32 reference (softmax weights are ~3e-7 here so the update is a tiny
correction on A; the append branch has 45% margin on min_d).
"""

import sys

sys.path.insert(0, "/opt/trn_rl_repo")

import numpy as np

import concourse.bass as bass
import concourse.bacc as bacc
import concourse.mybir as mybir
import concourse.tile as tile
from concourse import bass_isa
from concourse.bass_utils import run_bass_kernel_spmd

# ---- problem constants (hardcoded; kernel.py must be self-contained) ----
N_MEM = 2_000_000
D = 64
C = 10
OD = D + C  # 74
TIME_PERIOD = 100.0
COEF = 0.95
EMA = 2.0 / (TIME_PERIOD + 1.0)          # 0.019801980198019802
EMA_TEMP = 2.0 / (EMA + 1.0)             # ~1.9611650485436893

N_CORES = 8
P = 128


def make_cfg(n_real_rows, k_main=64, k_outer=32, n_cores=N_CORES, n_cache=16):
    """Static per-core tiling config."""
    rows_per_tile = P * k_main
    n_pad_rows = -(-n_real_rows // P) * P  # round up to 128
    n_main = n_pad_rows // rows_per_tile
    rem = n_pad_rows - n_main * rows_per_tile
    k_tail = rem // P
    assert n_main * rows_per_tile + k_tail * P == n_pad_rows
    return dict(
        n_real=n_real_rows,
        n_pad=n_pad_rows,
        k_main=k_main,
        k_outer=min(k_outer, k_main),
        n_main=n_main,
        k_tail=k_tail,
        n_cores=n_cores,
        n_cache=n_cache,
        n_e_cols=n_main * k_main + k_tail,
    )


def dsdm_kernel_body(tc, A, M, QA, QC, GE, IDT, WQBLK, QCBLK, OUT, cfg):
    """Emit the kernel IR. All-caps args are DRAM APs (per-core shard)."""
    nc = tc.nc
    f32 = mybir.dt.float32
    bf16 = mybir.dt.bfloat16
    K = cfg["k_main"]
    KT = cfg["k_tail"]
    n_main = cfg["n_main"]
    n_pad = cfg["n_pad"]
    NE = cfg["n_e_cols"]
    KO = cfg["k_outer"]
    n_grp = -(-K // KO)
    reps = cfg.get("reps", 1)
    PSUM = bass.MemorySpace.PSUM

    rows_main = n_main * P * K
    A_t = A[0:rows_main, :].rearrange("(t p k) d -> t p (k d)", p=P, k=K)
    M_t = M[0:rows_main, :].rearrange("(t p k) d -> t p (k d)", p=P, k=K)
    O_t = OUT[0:rows_main, :].rearrange("(t p k) d -> t p (k d)", p=P, k=K)
    if KT:
        A_tail = A[rows_main:n_pad, :].rearrange("(p k) d -> p (k d)", k=KT)
        M_tail = M[rows_main:n_pad, :].rearrange("(p k) d -> p (k d)", k=KT)
        O_tail = OUT[rows_main:n_pad, :].rearrange("(p k) d -> p (k d)", k=KT)
    else:
        A_tail = M_tail = O_tail = None

    with (
        tc.tile_pool(name="const", bufs=1) as cpool,
        tc.tile_pool(name="cache", bufs=1) as cachepool,
        tc.tile_pool(name="tin", bufs=3) as tpool,
        tc.tile_pool(name="mod", bufs=1) as opool,
        tc.tile_pool(name="m_in", bufs=2) as mpool,
        tc.tile_pool(name="bnc", bufs=2) as bpool,
        tc.tile_pool(name="ws", bufs=2) as wspool,
        tc.tile_pool(name="psT", bufs=2, space=PSUM) as psT,
        tc.tile_pool(name="psA", bufs=1, space=PSUM) as psA,
        tc.tile_pool(name="psM", bufs=2, space=PSUM) as psM,
        tc.tile_pool(name="dram", bufs=2, space="DRAM") as drampool,
    ):
        # ---------------- setup: replicated query tiles, persistent bufs ----
        # q replicated to every (partition, k) slot so the phase-A subtract is
        # a dense step-1 bf16 tensor_tensor (2x DVE mode, no broadcast AP).
        q_full = cpool.tile([P, K * D], bf16)
        qa_src = QA.rearrange("(o k d) -> o k d", o=1, k=1)  # [1, 1, 64]
        nc.gpsimd.dma_start(
            q_full.rearrange("p (k d) -> p k d", k=K),
            qa_src.broadcast_to((P, K, D)),
        )
        ge_sb = cpool.tile([1, 1], f32)
        nc.scalar.dma_start(ge_sb, GE.rearrange("(o d) -> o d", o=1))
        idt = cpool.tile([P, P], bf16)          # identity for PE transpose
        nc.gpsimd.dma_start(idt, IDT)
        # single-group block-diag q / qc at partition base 0 (every KO-group
        # uses the same rhs; only the lhsT w-transpose differs per group)
        wqblk = cpool.tile([KO, KO * D], bf16)
        nc.gpsimd.dma_start(wqblk, WQBLK)
        qcblk = cpool.tile([KO, KO * C], bf16)
        nc.gpsimd.dma_start(qcblk, QCBLK)

        # d_sb holds dist^2 -> dist -> exp(-dist) in place, [P, NE] f32
        d_sb = cpool.tile([P, NE], f32)
        scal = cpool.tile([P, 16], f32)
        gam_b = cpool.tile([P, 1], f32)
        ngam_b = cpool.tile([P, 1], f32)

        # persistent bf16 A-tile cache (the last n_cache tiles incl. tail)
        n_tiles = n_main + (1 if KT else 0)
        n_cache = min(cfg["n_cache"], n_tiles)
        cache = {}
        for t in range(n_tiles - n_cache, n_tiles):
            k = K if t < n_main else KT
            cache[t] = cachepool.tile([P, k * D], bf16, tag=f"c{t}",
                                      name=f"cache{t}")

        for _rep in range(reps):
            _dsdm_one_pass(
                tc, cfg, A_t, M_t, O_t, A_tail, M_tail, O_tail,
                q_full, idt, wqblk, qcblk, ge_sb, d_sb, scal, gam_b, ngam_b,
                cache, tpool, opool, mpool, bpool, wspool, psT, psA, psM,
                drampool, cpool,
            )


def _dsdm_one_pass(tc, cfg, A_t, M_t, O_t, A_tail, M_tail, O_tail, q_full,
                   idt, wqblk, qcblk, ge_sb, d_sb, scal, gam_b, ngam_b,
                   cache, tpool, opool, mpool, bpool, wspool, psT, psA, psM,
                   drampool, cpool):
    nc = tc.nc
    f32 = mybir.dt.float32
    bf16 = mybir.dt.bfloat16
    K = cfg["k_main"]
    KT = cfg["k_tail"]
    n_main = cfg["n_main"]
    NE = cfg["n_e_cols"]
    KO = cfg["k_outer"]
    n_tiles = n_main + (1 if KT else 0)
    n_cache = min(cfg["n_cache"], n_tiles)
    X = mybir.AxisListType.X
    ALU = mybir.AluOpType
    ACT = mybir.ActivationFunctionType
    sub_dve_frac = cfg.get("sub_dve_frac", 0.4)

    qf3 = q_full.rearrange("p (k d) -> p k d", k=K)

    # ---------------- phase A: dist^2 per row ---------------------------
    col = 0
    for t in range(n_tiles):
        k = K if t < n_main else KT
        a_dram = A_t[t] if t < n_main else A_tail
        cached = t in cache
        if cached:
            a = cache[t]
        else:
            a = tpool.tile([P, K * D], bf16, tag="t_in", name="a_in")
        nc.gpsimd.dma_start(a[:, : k * D], a_dram)  # f32 -> bf16 cast load
        if cached:
            tsub = tpool.tile([P, K * D], bf16, tag="t_in", name="t_sub")
        else:
            tsub = a  # uncached tiles are dead after the reduce: sub in place
        a3 = a.rearrange("p (k d) -> p k d", k=K) if k == K else \
            a[:, : k * D].rearrange("p (k d) -> p k d", k=k)
        t3 = tsub.rearrange("p (k d) -> p k d", k=K) if k == K else \
            tsub[:, : k * D].rearrange("p (k d) -> p k d", k=k)
        kv = max(1, min(k, round(sub_dve_frac * k)))
        nc.vector.tensor_sub(t3[:, :kv, :], a3[:, :kv, :], qf3[:, :kv, :])
        if kv < k:
            nc.gpsimd.tensor_sub(t3[:, kv:k, :], a3[:, kv:k, :],
                                 qf3[:, kv:k, :])
        nc.scalar.activation(tsub[:, : k * D], tsub[:, : k * D], ACT.Square)
        nc.vector.tensor_reduce(d_sb[:, col : col + k], t3[:, :k, :], axis=X,
                                op=ALU.add)
        col += k

    # ---------------- batched d=sqrt(d2), stats, e=exp(-d) in place -----
    nc.scalar.activation(d_sb[:, :NE], d_sb[:, :NE], ACT.Sqrt)  # now dist
    mloc = cpool.tile([P, 1], f32)
    nc.vector.tensor_reduce(mloc, d_sb[:, :NE], axis=X, op=ALU.min)
    sloc = cpool.tile([P, 1], f32)
    nc.scalar.activation(d_sb[:, :NE], d_sb[:, :NE], ACT.Exp, scale=-1.0,
                         accum_out=sloc)  # now exp(-dist)
    # Pad rows were filled host-side with a large constant: d ~ 8e5 so
    # exp(-d) underflows to exactly 0 and the min is unaffected.

    nmloc = cpool.tile([P, 1], f32)
    nc.vector.tensor_scalar_mul(nmloc, mloc, -1.0)
    nm_all = cpool.tile([P, 1], f32)
    s_all = cpool.tile([P, 1], f32)
    nc.gpsimd.partition_all_reduce(nm_all, nmloc, channels=P,
                                   reduce_op=bass_isa.ReduceOp.max)
    nc.gpsimd.partition_all_reduce(s_all, sloc, channels=P,
                                   reduce_op=bass_isa.ReduceOp.add)
    pack = cpool.tile([1, 8], f32)
    nc.vector.tensor_scalar_mul(pack[0:1, 0:1], nm_all[0:1, 0:1], -1.0)
    nc.vector.tensor_copy(pack[0:1, 1:2], s_all[0:1, 0:1])
    nc.vector.memset(pack[0:1, 2:8], 0.0)

    # ---------------- collective: AllGather the 8 (min, sum) pairs ------
    n_cores = cfg["n_cores"]
    if cfg.get("use_collective", True):
        cin = drampool.tile([1, 8], f32)
        cout = drampool.tile([n_cores, 8], f32)
        nc.sync.dma_start(cin, pack)
        nc.gpsimd.collective_compute(
            "AllGather",
            ALU.bypass,
            replica_groups=[list(range(n_cores))],
            ins=[cin[:, :].opt()],
            outs=[cout[:, :].opt()],
        )
        g8 = cpool.tile([n_cores, 8], f32)
        nc.sync.dma_start(g8, cout)

        ng = cpool.tile([n_cores, 1], f32)
        nc.vector.tensor_scalar_mul(ng, g8[:, 0:1], -1.0)
        ng_all = cpool.tile([n_cores, 1], f32)
        z_all = cpool.tile([n_cores, 1], f32)
        nc.gpsimd.partition_all_reduce(ng_all, ng, channels=n_cores,
                                       reduce_op=bass_isa.ReduceOp.max)
        nc.gpsimd.partition_all_reduce(z_all, g8[:, 1:2], channels=n_cores,
                                       reduce_op=bass_isa.ReduceOp.add)
    else:
        ng_all, z_all = nm_all, s_all  # single-core: locals are global

    # ---------------- scalar math on partition 0 ------------------------
    s0 = scal  # [P, 16] scratch; row 0 columns
    mstar = s0[0:1, 0:1]
    nc.vector.tensor_scalar_mul(mstar, ng_all[0:1, 0:1], -1.0)
    zrec = s0[0:1, 1:2]
    nc.vector.reciprocal(zrec, z_all[0:1, 0:1])
    t1 = s0[0:1, 2:3]
    nc.vector.tensor_scalar_mul(t1, mstar, float(EMA_TEMP))
    t2 = s0[0:1, 3:4]
    nc.vector.tensor_scalar_mul(t2, ge_sb, float(1.0 - EMA_TEMP))
    newge = s0[0:1, 4:5]
    nc.vector.tensor_add(newge, t1, t2)
    thr = s0[0:1, 5:6]
    nc.vector.tensor_scalar_mul(thr, newge, float(COEF))
    app = s0[0:1, 6:7]
    nc.vector.tensor_tensor(app, mstar, thr, op=ALU.is_ge)
    keep = s0[0:1, 7:8]
    nc.vector.tensor_scalar(keep, app, -1.0, 1.0, op0=ALU.mult, op1=ALU.add)
    gam1 = s0[0:1, 8:9]
    nc.vector.tensor_mul(gam1, keep, zrec)
    gam = s0[0:1, 9:10]
    nc.vector.tensor_scalar_mul(gam, gam1, float(EMA))
    nc.gpsimd.partition_broadcast(gam_b, gam, channels=P)
    nc.vector.tensor_scalar_mul(ngam_b, gam_b, -1.0)

    # ---------------- phase C: out = a*s + w*q (s = 1 - w, w = gam*e) ---
    # s and w are per-(row) scalars; PE replicates them across columns via
    # outer products (wT@qblk -> w*q, sT@onesblk -> s replicated), ACT
    # bounces PSUM->SBUF as bf16, DVE does two dense bf16 tensor_tensor ops.
    col = 0
    for t in range(n_tiles):
        k = K if t < n_main else KT
        m_dram = M_t[t] if t < n_main else M_tail
        o_dram = O_t[t] if t < n_main else O_tail
        cached = t in cache
        if cached:
            a = cache[t]
        else:
            a = tpool.tile([P, K * D], bf16, tag="t_in", name="a_rd")
            a_dram = A_t[t] if t < n_main else A_tail
            nc.gpsimd.dma_start(a[:, : k * D], a_dram)  # cast re-load
        m = mpool.tile([P, K * C], bf16, tag="m_in", name="m_in")
        nc.gpsimd.dma_start(m[:, : k * C], m_dram)  # cast load

        e_ap = d_sb[:, col : col + k]
        ws = wspool.tile([P, 2 * K], bf16, tag="ws", name="ws")
        w_ap = ws[:, 0:k]
        s_ap = ws[:, K : K + k]
        nc.vector.tensor_scalar_mul(w_ap, e_ap, gam_b[:, 0:1])  # w = gam*e
        nc.vector.tensor_scalar(s_ap, e_ap, ngam_b[:, 0:1], 1.0,
                                op0=ALU.mult, op1=ALU.add)       # s = 1-gam*e

        o = opool.tile([P, K * OD], bf16, tag="o", bufs=2, name="o_tile")
        o3 = o.rearrange("p (k d) -> p k d", k=K)
        a3 = a.rearrange("p (k d) -> p k d", k=K) if k == K else \
            a[:, : k * D].rearrange("p (k d) -> p k d", k=k)
        m3 = m.rearrange("p (k d) -> p k d", k=K) if k == K else \
            m[:, : k * C].rearrange("p (k d) -> p k d", k=k)
        s_bd = s_ap.to_broadcast((P, k, D))
        s_bc = s_ap.to_broadcast((P, k, C))
        # oA = a*s and oM = m*s on GPSIMD (broadcast multiply); the PE
        # outer-product corrections are added by DVE below.
        nc.gpsimd.tensor_tensor(o3[:, :k, 0:D], a3[:, :k, :], s_bd,
                                op=ALU.mult)
        nc.vector.tensor_tensor(o3[:, :k, D:OD], m3[:, :k, :], s_bc,
                                op=ALU.mult)

        for ko in range(0, k, KO):
            ks = min(KO, k - ko)
            wt_ps = psT.tile([KO, P], bf16, tag="wt")
            nc.tensor.transpose(wt_ps[:ks, :], w_ap[:, ko : ko + ks], idt)
            wt_sb = wspool.tile([KO, P], bf16, tag="wt_sb", name="wt_sb")
            nc.scalar.copy(wt_sb[:ks, :], wt_ps[:ks, :])
            pa = psA.tile([P, KO * D], f32, tag="pa")
            for j in range(0, ks * D, 512):
                je = min(j + 512, ks * D)
                nc.tensor.matmul(pa[:, j:je], wt_sb[:ks, :],
                                 wqblk[:ks, j:je], start=True, stop=True)
            pm = psM.tile([P, KO * C], f32, tag="pm")
            nc.tensor.matmul(pm[:, : ks * C], wt_sb[:ks, :],
                             qcblk[:ks, : ks * C], start=True, stop=True)
            pab = bpool.tile([P, KO * D], bf16, tag="pab", name="pab")
            nc.scalar.copy(pab[:, : ks * D], pa[:, : ks * D])
            pab3 = pab.rearrange("p (k d) -> p k d", k=KO)[:, :ks, :]
            oAs = o3[:, ko : ko + ks, 0:D]
            oMs = o3[:, ko : ko + ks, D:OD]
            nc.vector.tensor_tensor(oAs, oAs, pab3, op=ALU.add)    # += w*q
            pm3 = pm.rearrange("p (k d) -> p k d", k=KO)[:, :ks, :]
            nc.vector.tensor_tensor(oMs, oMs, pm3, op=ALU.add)     # += w*qc
        nc.gpsimd.dma_start(o_dram, o[:, : k * OD])  # bf16 -> f32 cast write
        col += k


_BUILD_CACHE = {}


def build_nc(cfg):
    key = tuple(sorted(cfg.items()))
    if key in _BUILD_CACHE:
        return _BUILD_CACHE[key]
    nc = bacc.Bacc("TRN2", target_bir_lowering=False, debug=False,
                   num_devices=cfg["n_cores"])
    f32 = mybir.dt.float32
    n_pad = cfg["n_pad"]
    K = cfg["k_main"]
    KO = cfg["k_outer"]
    n_grp = -(-K // KO)
    A = nc.dram_tensor("A", [n_pad, D], f32, kind="ExternalInput").ap()
    M = nc.dram_tensor("M", [n_pad, C], f32, kind="ExternalInput").ap()
    QA = nc.dram_tensor("QA", [D], f32, kind="ExternalInput").ap()
    QC = nc.dram_tensor("QC", [C], f32, kind="ExternalInput").ap()
    GE = nc.dram_tensor("GE", [1], f32, kind="ExternalInput").ap()
    IDT = nc.dram_tensor("IDT", [P, P], f32, kind="ExternalInput").ap()
    WQBLK = nc.dram_tensor("WQBLK", [KO, KO * D], f32,
                           kind="ExternalInput").ap()
    QCBLK = nc.dram_tensor("QCBLK", [KO, KO * C], f32,
                           kind="ExternalInput").ap()
    OUT = nc.dram_tensor("OUT", [n_pad, OD], f32, kind="ExternalOutput").ap()
    with tile.TileContext(nc) as tc:
        dsdm_kernel_body(tc, A, M, QA, QC, GE, IDT, WQBLK, QCBLK, OUT, cfg)
    nc.compile()
    _BUILD_CACHE[key] = nc
    return nc


PAD_VALUE = 1.0e4  # pad rows -> dist ~8e5 -> exp underflows to 0; min unaffected


def make_aux_inputs(cfg, qa, qc):
    """Host-built constants: identity + single-group block-diag q/qc."""
    k = cfg["k_outer"]
    qblk = np.zeros((k, k * D), np.float32)
    qcblk = np.zeros((k, k * C), np.float32)
    for kk in range(k):
        qblk[kk, kk * D : (kk + 1) * D] = qa
        qcblk[kk, kk * C : (kk + 1) * C] = qc
    return {
        "IDT": np.eye(P, dtype=np.float32),
        "WQBLK": qblk,
        "QCBLK": qcblk,
    }


def _shard_pad(x, n_cores, n_real, n_pad):
    """Split rows across cores, pad each shard to n_pad with PAD_VALUE rows."""
    shards = []
    pad = n_pad - n_real
    for c in range(n_cores):
        s = x[c * n_real : (c + 1) * n_real]
        if pad:
            s = np.concatenate(
                [s, np.full((pad, s.shape[1]), PAD_VALUE, dtype=np.float32)], axis=0
            )
        shards.append(np.ascontiguousarray(s, dtype=np.float32))
    return shards


_WARMED = False


def _warm_devices(n_cores, tries=7, wait=45.0):
    """Touch every core with a trivial op before the real run.

    The axon terminal occasionally reports NRT_EXEC_UNIT_UNRECOVERABLE on the
    first use after another session exited uncleanly, and recovers on its own
    within a couple of minutes — retry cheap ops until the mesh is healthy."""
    global _WARMED
    if _WARMED:
        return
    import time as _time

    import jax
    import jax.numpy as jnp

    last = None
    for t in range(tries):
        try:
            for d in jax.devices()[:n_cores]:
                y = jax.device_put(np.zeros(4, np.float32), d)
                assert float(jnp.sum(y).block_until_ready()) == 0.0
            _WARMED = True
            return
        except Exception as e:  # noqa: BLE001 - retry any backend error
            last = e
            _time.sleep(wait)
    raise RuntimeError(f"NeuronCores unavailable after {tries} tries") from last


def kernel(A, M, query_address, query_content, global_error, _trace=False):
    A = np.asarray(A, dtype=np.float32)
    M = np.asarray(M, dtype=np.float32)
    qa = np.ascontiguousarray(np.asarray(query_address, dtype=np.float32))
    qc = np.ascontiguousarray(np.asarray(query_content, dtype=np.float32))
    ge = np.ascontiguousarray(np.asarray(global_error, dtype=np.float32))

    n_total = A.shape[0]
    n_cores = N_CORES
    assert n_total % n_cores == 0
    n_real = n_total // n_cores
    cfg = make_cfg(n_real)
    nc = build_nc(cfg)
    _warm_devices(n_cores)

    a_sh = _shard_pad(A, n_cores, n_real, cfg["n_pad"])
    m_sh = _shard_pad(M, n_cores, n_real, cfg["n_pad"])
    aux = make_aux_inputs(cfg, qa, qc)
    in_maps = [
        {"A": a_sh[c], "M": m_sh[c], "QA": qa, "QC": qc, "GE": ge, **aux}
        for c in range(n_cores)
    ]
    res = run_bass_kernel_spmd(nc, in_maps, core_ids=list(range(n_cores)),
                               trace=False)
    outs = [res.results[c]["OUT"][:n_real] for c in range(n_cores)]
    full = np.concatenate(outs, axis=0)
    if _trace:
        kernel.last_results = res
    return full


if __name__ == "__main__":
    # smoke test with random data (no reference comparison here)
    rng = np.random.default_rng(0)
    A = rng.standard_normal((N_MEM, D), dtype=np.float32)
    M = rng.standard_normal((N_MEM, C), dtype=np.float32)
    qa = rng.standard_normal(D).astype(np.float32)
    qc = rng.standard_normal(C).astype(np.float32)
    ge = rng.random(1, dtype=np.float32)
    out = kernel(A, M, qa, qc, ge)
    print("out", out.shape, out.dtype, float(np.abs(out).max()))


# revision 15
# speedup vs baseline: 1.3157x; 1.3157x over previous
"""DSDM memory-update kernel for Trainium2, SPMD across 8 NeuronCores.

Computation (per reference):
    d_i   = ||A_i - q_a||_2                      (i over 2M rows)
    min_d = min_i d_i
    new_ge = ge + ema_temp*(min_d - ge)
    append = min_d >= 0.95*new_ge
    w_i   = softmax(-d)_i * ema   (zeroed when append)
    A'    = A + w*(q_a - A);  M' = M + w*(q_c - M)
    out   = concat([A', M'], axis=1)

v2 strategy (memory-regime): the kernel is HBM-bound (A read twice + M read +
OUT written = 212 MB/core at ~358 GB/s). Cut traffic by loading A as bf16 via
SWDGE cast-DMA and keeping `n_cache` of the 31 row-tiles resident in SBUF
between the distance pass and the update pass; only the rest are re-read.
All heavy elementwise runs in bf16 (2x DVE mode), the per-row scales are
replicated across columns on the idle PE (outer products w^T@qblk and
s^T@onesblk), and OUT is staged bf16 in SBUF and cast-written to f32 by the
DMA. Verified numerically: bf16 arithmetic gives max rel err ~0.6% vs the
f32 reference (softmax weights are ~3e-7 here so the update is a tiny
correction on A; the append branch has 45% margin on min_d).
"""

import sys

sys.path.insert(0, "/opt/trn_rl_repo")

import numpy as np

import concourse.bass as bass
import concourse.bacc as bacc
import concourse.mybir as mybir
import concourse.tile as tile
from concourse import bass_isa
from concourse.bass_utils import run_bass_kernel_spmd

# ---- problem constants (hardcoded; kernel.py must be self-contained) ----
N_MEM = 2_000_000
D = 64
C = 10
OD = D + C  # 74
TIME_PERIOD = 100.0
COEF = 0.95
EMA = 2.0 / (TIME_PERIOD + 1.0)          # 0.019801980198019802
EMA_TEMP = 2.0 / (EMA + 1.0)             # ~1.9611650485436893

N_CORES = 8
P = 128


def make_cfg(n_real_rows, k_main=64, k_outer=32, n_cores=N_CORES, n_cache=16):
    """Static per-core tiling config."""
    rows_per_tile = P * k_main
    n_pad_rows = -(-n_real_rows // P) * P  # round up to 128
    n_main = n_pad_rows // rows_per_tile
    rem = n_pad_rows - n_main * rows_per_tile
    k_tail = rem // P
    assert n_main * rows_per_tile + k_tail * P == n_pad_rows
    return dict(
        n_real=n_real_rows,
        n_pad=n_pad_rows,
        k_main=k_main,
        k_outer=min(k_outer, k_main),
        n_main=n_main,
        k_tail=k_tail,
        n_cores=n_cores,
        n_cache=n_cache,
        n_e_cols=n_main * k_main + k_tail,
    )


def dsdm_kernel_body(tc, A, M, QA, QC, GE, IDT, WQBLK, QCBLK, ONEBLK, OUT, cfg):
    """Emit the kernel IR. All-caps args are DRAM APs (per-core shard)."""
    nc = tc.nc
    f32 = mybir.dt.float32
    bf16 = mybir.dt.bfloat16
    K = cfg["k_main"]
    KT = cfg["k_tail"]
    n_main = cfg["n_main"]
    n_pad = cfg["n_pad"]
    NE = cfg["n_e_cols"]
    KO = cfg["k_outer"]
    n_grp = -(-K // KO)
    reps = cfg.get("reps", 1)
    PSUM = bass.MemorySpace.PSUM

    rows_main = n_main * P * K
    A_t = A[0:rows_main, :].rearrange("(t p k) d -> t p (k d)", p=P, k=K)
    M_t = M[0:rows_main, :].rearrange("(t p k) d -> t p (k d)", p=P, k=K)
    O_t = OUT[0:rows_main, :].rearrange("(t p k) d -> t p (k d)", p=P, k=K)
    if KT:
        A_tail = A[rows_main:n_pad, :].rearrange("(p k) d -> p (k d)", k=KT)
        M_tail = M[rows_main:n_pad, :].rearrange("(p k) d -> p (k d)", k=KT)
        O_tail = OUT[rows_main:n_pad, :].rearrange("(p k) d -> p (k d)", k=KT)
    else:
        A_tail = M_tail = O_tail = None

    with (
        tc.tile_pool(name="const", bufs=1) as cpool,
        tc.tile_pool(name="cache", bufs=1) as cachepool,
        tc.tile_pool(name="tin", bufs=3) as tpool,
        tc.tile_pool(name="mod", bufs=1) as opool,
        tc.tile_pool(name="m_in", bufs=2) as mpool,
        tc.tile_pool(name="bnc", bufs=2) as bpool,
        tc.tile_pool(name="ws", bufs=2) as wspool,
        tc.tile_pool(name="psT", bufs=2, space=PSUM) as psT,
        tc.tile_pool(name="psA", bufs=1, space=PSUM) as psA,
        tc.tile_pool(name="psM", bufs=2, space=PSUM) as psM,
        tc.tile_pool(name="dram", bufs=2, space="DRAM") as drampool,
    ):
        # ---------------- setup: replicated query tiles, persistent bufs ----
        # q replicated to every (partition, k) slot so the phase-A subtract is
        # a dense step-1 bf16 tensor_tensor (2x DVE mode, no broadcast AP).
        q_full = cpool.tile([P, K * D], bf16)
        qa_src = QA.rearrange("(o k d) -> o k d", o=1, k=1)  # [1, 1, 64]
        nc.gpsimd.dma_start(
            q_full.rearrange("p (k d) -> p k d", k=K),
            qa_src.broadcast_to((P, K, D)),
        )
        ge_sb = cpool.tile([1, 1], f32)
        nc.scalar.dma_start(ge_sb, GE.rearrange("(o d) -> o d", o=1))
        idt = cpool.tile([P, P], bf16)          # identity for PE transpose
        nc.gpsimd.dma_start(idt, IDT)
        # single-group block-diag q / qc at partition base 0 (every KO-group
        # uses the same rhs; only the lhsT w-transpose differs per group)
        wqblk = cpool.tile([KO, KO * D], bf16)
        nc.gpsimd.dma_start(wqblk, WQBLK)
        qcblk = cpool.tile([KO, KO * C], bf16)
        nc.gpsimd.dma_start(qcblk, QCBLK)
        if cfg.get("oa_mult_engine", "gpsimd") == "pe":
            oneblk = cpool.tile([KO, KO * D], bf16)
            nc.gpsimd.dma_start(oneblk, ONEBLK)
        else:
            oneblk = None

        # d_sb holds dist^2 -> dist -> exp(-dist) in place, [P, NE] f32
        d_sb = cpool.tile([P, NE], f32)
        scal = cpool.tile([P, 16], f32)
        gam_b = cpool.tile([P, 1], f32)
        ngam_b = cpool.tile([P, 1], f32)

        # persistent bf16 A-tile cache (the last n_cache tiles incl. tail)
        n_tiles = n_main + (1 if KT else 0)
        n_cache = min(cfg["n_cache"], n_tiles)
        cache = {}
        for t in range(n_tiles - n_cache, n_tiles):
            k = K if t < n_main else KT
            cache[t] = cachepool.tile([P, k * D], bf16, tag=f"c{t}",
                                      name=f"cache{t}")

        for _rep in range(reps):
            _dsdm_one_pass(
                tc, cfg, A_t, M_t, O_t, A_tail, M_tail, O_tail,
                q_full, idt, wqblk, qcblk, oneblk, ge_sb, d_sb, scal, gam_b,
                ngam_b, cache, tpool, opool, mpool, bpool, wspool, psT, psA,
                psM, drampool, cpool,
            )


def _dsdm_one_pass(tc, cfg, A_t, M_t, O_t, A_tail, M_tail, O_tail, q_full,
                   idt, wqblk, qcblk, oneblk, ge_sb, d_sb, scal, gam_b,
                   ngam_b, cache, tpool, opool, mpool, bpool, wspool, psT,
                   psA, psM, drampool, cpool):
    nc = tc.nc
    f32 = mybir.dt.float32
    bf16 = mybir.dt.bfloat16
    K = cfg["k_main"]
    KT = cfg["k_tail"]
    n_main = cfg["n_main"]
    NE = cfg["n_e_cols"]
    KO = cfg["k_outer"]
    n_tiles = n_main + (1 if KT else 0)
    n_cache = min(cfg["n_cache"], n_tiles)
    X = mybir.AxisListType.X
    ALU = mybir.AluOpType
    ACT = mybir.ActivationFunctionType
    sub_dve_frac = cfg.get("sub_dve_frac", 0.4)

    qf3 = q_full.rearrange("p (k d) -> p k d", k=K)

    # ---------------- phase A: dist^2 per row ---------------------------
    col = 0
    for t in range(n_tiles):
        k = K if t < n_main else KT
        a_dram = A_t[t] if t < n_main else A_tail
        cached = t in cache
        if cached:
            a = cache[t]
        else:
            a = tpool.tile([P, K * D], bf16, tag="t_in", name="a_in")
        nc.gpsimd.dma_start(a[:, : k * D], a_dram)  # f32 -> bf16 cast load
        if cached:
            tsub = tpool.tile([P, K * D], bf16, tag="t_in", name="t_sub")
        else:
            tsub = a  # uncached tiles are dead after the reduce: sub in place
        a3 = a.rearrange("p (k d) -> p k d", k=K) if k == K else \
            a[:, : k * D].rearrange("p (k d) -> p k d", k=k)
        t3 = tsub.rearrange("p (k d) -> p k d", k=K) if k == K else \
            tsub[:, : k * D].rearrange("p (k d) -> p k d", k=k)
        kv = max(1, min(k, round(sub_dve_frac * k)))
        nc.vector.tensor_sub(t3[:, :kv, :], a3[:, :kv, :], qf3[:, :kv, :])
        if kv < k:
            nc.gpsimd.tensor_sub(t3[:, kv:k, :], a3[:, kv:k, :],
                                 qf3[:, kv:k, :])
        nc.scalar.activation(tsub[:, : k * D], tsub[:, : k * D], ACT.Square)
        nc.vector.tensor_reduce(d_sb[:, col : col + k], t3[:, :k, :], axis=X,
                                op=ALU.add)
        col += k

    # ---------------- batched d=sqrt(d2), stats, e=exp(-d) in place -----
    nc.scalar.activation(d_sb[:, :NE], d_sb[:, :NE], ACT.Sqrt)  # now dist
    mloc = cpool.tile([P, 1], f32)
    nc.vector.tensor_reduce(mloc, d_sb[:, :NE], axis=X, op=ALU.min)
    sloc = cpool.tile([P, 1], f32)
    nc.scalar.activation(d_sb[:, :NE], d_sb[:, :NE], ACT.Exp, scale=-1.0,
                         accum_out=sloc)  # now exp(-dist)
    # Pad rows were filled host-side with a large constant: d ~ 8e5 so
    # exp(-d) underflows to exactly 0 and the min is unaffected.

    nmloc = cpool.tile([P, 1], f32)
    nc.vector.tensor_scalar_mul(nmloc, mloc, -1.0)
    nm_all = cpool.tile([P, 1], f32)
    s_all = cpool.tile([P, 1], f32)
    nc.gpsimd.partition_all_reduce(nm_all, nmloc, channels=P,
                                   reduce_op=bass_isa.ReduceOp.max)
    nc.gpsimd.partition_all_reduce(s_all, sloc, channels=P,
                                   reduce_op=bass_isa.ReduceOp.add)
    pack = cpool.tile([1, 8], f32)
    nc.vector.tensor_scalar_mul(pack[0:1, 0:1], nm_all[0:1, 0:1], -1.0)
    nc.vector.tensor_copy(pack[0:1, 1:2], s_all[0:1, 0:1])
    nc.vector.memset(pack[0:1, 2:8], 0.0)

    # ---------------- collective: AllGather the 8 (min, sum) pairs ------
    n_cores = cfg["n_cores"]
    if cfg.get("use_collective", True):
        cin = drampool.tile([1, 8], f32)
        cout = drampool.tile([n_cores, 8], f32)
        nc.sync.dma_start(cin, pack)
        nc.gpsimd.collective_compute(
            "AllGather",
            ALU.bypass,
            replica_groups=[list(range(n_cores))],
            ins=[cin[:, :].opt()],
            outs=[cout[:, :].opt()],
        )
        g8 = cpool.tile([n_cores, 8], f32)
        nc.sync.dma_start(g8, cout)

        ng = cpool.tile([n_cores, 1], f32)
        nc.vector.tensor_scalar_mul(ng, g8[:, 0:1], -1.0)
        ng_all = cpool.tile([n_cores, 1], f32)
        z_all = cpool.tile([n_cores, 1], f32)
        nc.gpsimd.partition_all_reduce(ng_all, ng, channels=n_cores,
                                       reduce_op=bass_isa.ReduceOp.max)
        nc.gpsimd.partition_all_reduce(z_all, g8[:, 1:2], channels=n_cores,
                                       reduce_op=bass_isa.ReduceOp.add)
    else:
        ng_all, z_all = nm_all, s_all  # single-core: locals are global

    # ---------------- scalar math on partition 0 ------------------------
    s0 = scal  # [P, 16] scratch; row 0 columns
    mstar = s0[0:1, 0:1]
    nc.vector.tensor_scalar_mul(mstar, ng_all[0:1, 0:1], -1.0)
    zrec = s0[0:1, 1:2]
    nc.vector.reciprocal(zrec, z_all[0:1, 0:1])
    t1 = s0[0:1, 2:3]
    nc.vector.tensor_scalar_mul(t1, mstar, float(EMA_TEMP))
    t2 = s0[0:1, 3:4]
    nc.vector.tensor_scalar_mul(t2, ge_sb, float(1.0 - EMA_TEMP))
    newge = s0[0:1, 4:5]
    nc.vector.tensor_add(newge, t1, t2)
    thr = s0[0:1, 5:6]
    nc.vector.tensor_scalar_mul(thr, newge, float(COEF))
    app = s0[0:1, 6:7]
    nc.vector.tensor_tensor(app, mstar, thr, op=ALU.is_ge)
    keep = s0[0:1, 7:8]
    nc.vector.tensor_scalar(keep, app, -1.0, 1.0, op0=ALU.mult, op1=ALU.add)
    gam1 = s0[0:1, 8:9]
    nc.vector.tensor_mul(gam1, keep, zrec)
    gam = s0[0:1, 9:10]
    nc.vector.tensor_scalar_mul(gam, gam1, float(EMA))
    nc.gpsimd.partition_broadcast(gam_b, gam, channels=P)
    nc.vector.tensor_scalar_mul(ngam_b, gam_b, -1.0)

    if cfg.get("skip_phase_c"):
        return

    # ---------------- phase C: out = a*s + w*q (s = 1 - w, w = gam*e) ---
    # s and w are per-(row) scalars; PE replicates them across columns via
    # outer products (wT@qblk -> w*q, sT@onesblk -> s replicated), ACT
    # bounces PSUM->SBUF as bf16, DVE does two dense bf16 tensor_tensor ops.
    col = 0
    for t in range(n_tiles):
        k = K if t < n_main else KT
        m_dram = M_t[t] if t < n_main else M_tail
        o_dram = O_t[t] if t < n_main else O_tail
        cached = t in cache
        if cached:
            a = cache[t]
        else:
            a = tpool.tile([P, K * D], bf16, tag="t_in", name="a_rd")
            a_dram = A_t[t] if t < n_main else A_tail
            nc.gpsimd.dma_start(a[:, : k * D], a_dram)  # cast re-load
        m = mpool.tile([P, K * C], bf16, tag="m_in", name="m_in")
        nc.gpsimd.dma_start(m[:, : k * C], m_dram)  # cast load

        e_ap = d_sb[:, col : col + k]
        ws = wspool.tile([P, 2 * K], bf16, tag="ws", name="ws")
        w_ap = ws[:, 0:k]
        s_ap = ws[:, K : K + k]
        nc.vector.tensor_scalar_mul(w_ap, e_ap, gam_b[:, 0:1])  # w = gam*e
        nc.vector.tensor_scalar(s_ap, e_ap, ngam_b[:, 0:1], 1.0,
                                op0=ALU.mult, op1=ALU.add)       # s = 1-gam*e

        o = opool.tile([P, K * OD], bf16, tag="o", bufs=2, name="o_tile")
        o3 = o.rearrange("p (k d) -> p k d", k=K)
        a3 = a.rearrange("p (k d) -> p k d", k=K) if k == K else \
            a[:, : k * D].rearrange("p (k d) -> p k d", k=k)
        m3 = m.rearrange("p (k d) -> p k d", k=K) if k == K else \
            m[:, : k * C].rearrange("p (k d) -> p k d", k=k)
        s_bd = s_ap.to_broadcast((P, k, D))
        s_bc = s_ap.to_broadcast((P, k, C))
        # oA = a*s and oM = m*s (broadcast multiply); the PE outer-product
        # corrections are added by DVE below.
        oa_eng = cfg.get("oa_mult_engine", "gpsimd")
        if oa_eng == "gpsimd":
            nc.gpsimd.tensor_tensor(o3[:, :k, 0:D], a3[:, :k, :], s_bd,
                                    op=ALU.mult)
            nc.vector.tensor_tensor(o3[:, :k, D:OD], m3[:, :k, :], s_bc,
                                    op=ALU.mult)
        elif oa_eng == "pe":
            # a*s happens per KO-group below (s replicated on the PE)
            nc.gpsimd.tensor_tensor(o3[:, :k, D:OD], m3[:, :k, :], s_bc,
                                    op=ALU.mult)
        elif oa_eng == "split":
            kh = k // 2
            nc.gpsimd.tensor_tensor(o3[:, :kh, 0:D], a3[:, :kh, :],
                                    s_ap.to_broadcast((P, kh, D)), op=ALU.mult)
            nc.vector.tensor_tensor(o3[:, kh:k, 0:D], a3[:, kh:k, :],
                                    s_ap[:, kh:].to_broadcast((P, k - kh, D)),
                                    op=ALU.mult)
            nc.gpsimd.tensor_tensor(o3[:, :k, D:OD], m3[:, :k, :], s_bc,
                                    op=ALU.mult)
        else:  # dve
            nc.vector.tensor_tensor(o3[:, :k, 0:D], a3[:, :k, :], s_bd,
                                    op=ALU.mult)
            nc.gpsimd.tensor_tensor(o3[:, :k, D:OD], m3[:, :k, :], s_bc,
                                    op=ALU.mult)

        use_pe_srep = oa_eng == "pe"
        for ko in range(0, k, KO):
            ks = min(KO, k - ko)
            wt_ps = psT.tile([KO, P], bf16, tag="wt",
                             bufs=1 if use_pe_srep else 2)
            nc.tensor.transpose(wt_ps[:ks, :], w_ap[:, ko : ko + ks], idt)
            wt_sb = wspool.tile([KO, P], bf16, tag="wt_sb", name="wt_sb")
            nc.scalar.copy(wt_sb[:ks, :], wt_ps[:ks, :])
            pa = psA.tile([P, KO * D], f32, tag="pa", bufs=1)
            for j in range(0, ks * D, 512):
                je = min(j + 512, ks * D)
                nc.tensor.matmul(pa[:, j:je], wt_sb[:ks, :],
                                 wqblk[:ks, j:je], start=True, stop=True)
            pm = psM.tile([P, KO * C], f32, tag="pm",
                          bufs=1 if use_pe_srep else 2)
            nc.tensor.matmul(pm[:, : ks * C], wt_sb[:ks, :],
                             qcblk[:ks, : ks * C], start=True, stop=True)
            pab = bpool.tile([P, KO * D], bf16, tag="pab", name="pab")
            nc.scalar.copy(pab[:, : ks * D], pa[:, : ks * D])
            pab3 = pab.rearrange("p (k d) -> p k d", k=KO)[:, :ks, :]
            oAs = o3[:, ko : ko + ks, 0:D]
            oMs = o3[:, ko : ko + ks, D:OD]
            if use_pe_srep:
                # s replicated across D via PE outer product w^T @ onesblk;
                # the (1 - x) fold happens in the ACT bounce (scale/bias).
                pw = psA.tile([P, KO * D], f32, tag="pw", bufs=1)
                for j in range(0, ks * D, 512):
                    je = min(j + 512, ks * D)
                    nc.tensor.matmul(pw[:, j:je], wt_sb[:ks, :],
                                     oneblk[:ks, j:je], start=True, stop=True)
                psb = bpool.tile([P, KO * D], bf16, tag="psb", name="psb")
                nc.scalar.activation(psb[:, : ks * D], pw[:, : ks * D],
                                     ACT.Copy, bias=1.0, scale=-1.0)
                psb3 = psb.rearrange("p (k d) -> p k d", k=KO)[:, :ks, :]
                a3s = a3[:, ko : ko + ks, :]
                nc.vector.tensor_tensor(oAs, a3s, psb3, op=ALU.mult)  # a*s
            nc.vector.tensor_tensor(oAs, oAs, pab3, op=ALU.add)    # += w*q
            pm3 = pm.rearrange("p (k d) -> p k d", k=KO)[:, :ks, :]
            nc.vector.tensor_tensor(oMs, oMs, pm3, op=ALU.add)     # += w*qc
        nc.gpsimd.dma_start(o_dram, o[:, : k * OD])  # bf16 -> f32 cast write
        col += k


_BUILD_CACHE = {}


def build_nc(cfg):
    key = tuple(sorted(cfg.items()))
    if key in _BUILD_CACHE:
        return _BUILD_CACHE[key]
    nc = bacc.Bacc("TRN2", target_bir_lowering=False, debug=False,
                   num_devices=cfg["n_cores"])
    f32 = mybir.dt.float32
    n_pad = cfg["n_pad"]
    K = cfg["k_main"]
    KO = cfg["k_outer"]
    n_grp = -(-K // KO)
    A = nc.dram_tensor("A", [n_pad, D], f32, kind="ExternalInput").ap()
    M = nc.dram_tensor("M", [n_pad, C], f32, kind="ExternalInput").ap()
    QA = nc.dram_tensor("QA", [D], f32, kind="ExternalInput").ap()
    QC = nc.dram_tensor("QC", [C], f32, kind="ExternalInput").ap()
    GE = nc.dram_tensor("GE", [1], f32, kind="ExternalInput").ap()
    IDT = nc.dram_tensor("IDT", [P, P], f32, kind="ExternalInput").ap()
    WQBLK = nc.dram_tensor("WQBLK", [KO, KO * D], f32,
                           kind="ExternalInput").ap()
    ONEBLK = nc.dram_tensor("ONEBLK", [KO, KO * D], f32,
                            kind="ExternalInput").ap()
    QCBLK = nc.dram_tensor("QCBLK", [KO, KO * C], f32,
                           kind="ExternalInput").ap()
    OUT = nc.dram_tensor("OUT", [n_pad, OD], f32, kind="ExternalOutput").ap()
    with tile.TileContext(nc) as tc:
        dsdm_kernel_body(tc, A, M, QA, QC, GE, IDT, WQBLK, QCBLK, ONEBLK, OUT, cfg)
    nc.compile()
    _BUILD_CACHE[key] = nc
    return nc


PAD_VALUE = 1.0e4  # pad rows -> dist ~8e5 -> exp underflows to 0; min unaffected


def make_aux_inputs(cfg, qa, qc):
    """Host-built constants: identity + single-group block-diag q/qc."""
    k = cfg["k_outer"]
    qblk = np.zeros((k, k * D), np.float32)
    oblk = np.zeros((k, k * D), np.float32)
    qcblk = np.zeros((k, k * C), np.float32)
    for kk in range(k):
        qblk[kk, kk * D : (kk + 1) * D] = qa
        oblk[kk, kk * D : (kk + 1) * D] = 1.0
        qcblk[kk, kk * C : (kk + 1) * C] = qc
    return {
        "IDT": np.eye(P, dtype=np.float32),
        "WQBLK": qblk,
        "ONEBLK": oblk,
        "QCBLK": qcblk,
    }


def _shard_pad(x, n_cores, n_real, n_pad):
    """Split rows across cores, pad each shard to n_pad with PAD_VALUE rows."""
    shards = []
    pad = n_pad - n_real
    for c in range(n_cores):
        s = x[c * n_real : (c + 1) * n_real]
        if pad:
            s = np.concatenate(
                [s, np.full((pad, s.shape[1]), PAD_VALUE, dtype=np.float32)], axis=0
            )
        shards.append(np.ascontiguousarray(s, dtype=np.float32))
    return shards


_WARMED = False


def _warm_devices(n_cores, tries=7, wait=45.0):
    """Touch every core with a trivial op before the real run.

    The axon terminal occasionally reports NRT_EXEC_UNIT_UNRECOVERABLE on the
    first use after another session exited uncleanly, and recovers on its own
    within a couple of minutes — retry cheap ops until the mesh is healthy."""
    global _WARMED
    if _WARMED:
        return
    import time as _time

    import jax
    import jax.numpy as jnp

    last = None
    for t in range(tries):
        try:
            for d in jax.devices()[:n_cores]:
                y = jax.device_put(np.zeros(4, np.float32), d)
                assert float(jnp.sum(y).block_until_ready()) == 0.0
            _WARMED = True
            return
        except Exception as e:  # noqa: BLE001 - retry any backend error
            last = e
            _time.sleep(wait)
    raise RuntimeError(f"NeuronCores unavailable after {tries} tries") from last


def kernel(A, M, query_address, query_content, global_error, _trace=False):
    A = np.asarray(A, dtype=np.float32)
    M = np.asarray(M, dtype=np.float32)
    qa = np.ascontiguousarray(np.asarray(query_address, dtype=np.float32))
    qc = np.ascontiguousarray(np.asarray(query_content, dtype=np.float32))
    ge = np.ascontiguousarray(np.asarray(global_error, dtype=np.float32))

    n_total = A.shape[0]
    n_cores = N_CORES
    assert n_total % n_cores == 0
    n_real = n_total // n_cores
    cfg = make_cfg(n_real)
    nc = build_nc(cfg)
    _warm_devices(n_cores)

    a_sh = _shard_pad(A, n_cores, n_real, cfg["n_pad"])
    m_sh = _shard_pad(M, n_cores, n_real, cfg["n_pad"])
    aux = make_aux_inputs(cfg, qa, qc)
    in_maps = [
        {"A": a_sh[c], "M": m_sh[c], "QA": qa, "QC": qc, "GE": ge, **aux}
        for c in range(n_cores)
    ]
    res = run_bass_kernel_spmd(nc, in_maps, core_ids=list(range(n_cores)),
                               trace=False)
    outs = [res.results[c]["OUT"][:n_real] for c in range(n_cores)]
    full = np.concatenate(outs, axis=0)
    if _trace:
        kernel.last_results = res
    return full


if __name__ == "__main__":
    # smoke test with random data (no reference comparison here)
    rng = np.random.default_rng(0)
    A = rng.standard_normal((N_MEM, D), dtype=np.float32)
    M = rng.standard_normal((N_MEM, C), dtype=np.float32)
    qa = rng.standard_normal(D).astype(np.float32)
    qc = rng.standard_normal(C).astype(np.float32)
    ge = rng.random(1, dtype=np.float32)
    out = kernel(A, M, qa, qc, ge)
    print("out", out.shape, out.dtype, float(np.abs(out).max()))


# revision 19
# speedup vs baseline: 2.0975x; 1.5943x over previous
"""DSDM memory-update kernel for Trainium2, SPMD across 8 NeuronCores.

Computation (per reference):
    d_i   = ||A_i - q_a||_2                      (i over 2M rows)
    min_d = min_i d_i
    new_ge = ge + ema_temp*(min_d - ge)
    append = min_d >= 0.95*new_ge
    w_i   = softmax(-d)_i * ema   (zeroed when append)
    A'    = A + w*(q_a - A);  M' = M + w*(q_c - M)
    out   = concat([A', M'], axis=1)

v2 strategy (memory-regime): the kernel is HBM-bound (A read twice + M read +
OUT written = 212 MB/core at ~358 GB/s). Cut traffic by loading A as bf16 via
SWDGE cast-DMA and keeping `n_cache` of the 31 row-tiles resident in SBUF
between the distance pass and the update pass; only the rest are re-read.
All heavy elementwise runs in bf16 (2x DVE mode), the per-row scales are
replicated across columns on the idle PE (outer products w^T@qblk and
s^T@onesblk), and OUT is staged bf16 in SBUF and cast-written to f32 by the
DMA. Verified numerically: bf16 arithmetic gives max rel err ~0.6% vs the
f32 reference (softmax weights are ~3e-7 here so the update is a tiny
correction on A; the append branch has 45% margin on min_d).
"""

import sys

sys.path.insert(0, "/opt/trn_rl_repo")

import numpy as np

import concourse.bass as bass
import concourse.bacc as bacc
import concourse.mybir as mybir
import concourse.tile as tile
from concourse import bass_isa
from concourse.bass_utils import run_bass_kernel_spmd

# ---- problem constants (hardcoded; kernel.py must be self-contained) ----
N_MEM = 2_000_000
D = 64
C = 10
OD = D + C  # 74
TIME_PERIOD = 100.0
COEF = 0.95
EMA = 2.0 / (TIME_PERIOD + 1.0)          # 0.019801980198019802
EMA_TEMP = 2.0 / (EMA + 1.0)             # ~1.9611650485436893

N_CORES = 8
P = 128


def make_cfg(n_real_rows, k_main=64, k_outer=32, n_cores=N_CORES, n_cache=16):
    """Static per-core tiling config."""
    rows_per_tile = P * k_main
    n_pad_rows = -(-n_real_rows // P) * P  # round up to 128
    n_main = n_pad_rows // rows_per_tile
    rem = n_pad_rows - n_main * rows_per_tile
    k_tail = rem // P
    assert n_main * rows_per_tile + k_tail * P == n_pad_rows
    return dict(
        n_real=n_real_rows,
        n_pad=n_pad_rows,
        k_main=k_main,
        k_outer=min(k_outer, k_main),
        n_main=n_main,
        k_tail=k_tail,
        n_cores=n_cores,
        n_cache=n_cache,
        n_e_cols=n_main * k_main + k_tail,
    )


def dsdm_kernel_body(tc, A, M, QA, QC, GE, IDT, WQBLK, QCBLK, ONEBLK, OUT, cfg):
    """Emit the kernel IR. All-caps args are DRAM APs (per-core shard)."""
    nc = tc.nc
    f32 = mybir.dt.float32
    bf16 = mybir.dt.bfloat16
    K = cfg["k_main"]
    KT = cfg["k_tail"]
    n_main = cfg["n_main"]
    n_pad = cfg["n_pad"]
    NE = cfg["n_e_cols"]
    KO = cfg["k_outer"]
    n_grp = -(-K // KO)
    reps = cfg.get("reps", 1)
    PSUM = bass.MemorySpace.PSUM

    rows_main = n_main * P * K
    A_t = A[0:rows_main, :].rearrange("(t p k) d -> t p (k d)", p=P, k=K)
    M_t = M[0:rows_main, :].rearrange("(t p k) d -> t p (k d)", p=P, k=K)
    O_t = OUT[0:rows_main, :].rearrange("(t p k) d -> t p (k d)", p=P, k=K)
    if KT:
        A_tail = A[rows_main:n_pad, :].rearrange("(p k) d -> p (k d)", k=KT)
        M_tail = M[rows_main:n_pad, :].rearrange("(p k) d -> p (k d)", k=KT)
        O_tail = OUT[rows_main:n_pad, :].rearrange("(p k) d -> p (k d)", k=KT)
    else:
        A_tail = M_tail = O_tail = None

    with (
        tc.tile_pool(name="const", bufs=1) as cpool,
        tc.tile_pool(name="cache", bufs=1) as cachepool,
        tc.tile_pool(name="tin", bufs=3) as tpool,
        tc.tile_pool(name="mod", bufs=1) as opool,
        tc.tile_pool(name="m_in", bufs=2) as mpool,
        tc.tile_pool(name="bnc", bufs=2) as bpool,
        tc.tile_pool(name="ws", bufs=2) as wspool,
        tc.tile_pool(name="psT", bufs=2, space=PSUM) as psT,
        tc.tile_pool(name="psA", bufs=1, space=PSUM) as psA,
        tc.tile_pool(name="psM", bufs=2, space=PSUM) as psM,
        tc.tile_pool(name="dram", bufs=2, space="DRAM") as drampool,
    ):
        # ---------------- setup: replicated query tiles, persistent bufs ----
        # q replicated to every (partition, k) slot so the phase-A subtract is
        # a dense step-1 bf16 tensor_tensor (2x DVE mode, no broadcast AP).
        KQ = 1 if cfg.get("q_bcast") else K
        q_full = cpool.tile([P, KQ * D], bf16)
        qa_src = QA.rearrange("(o k d) -> o k d", o=1, k=1)  # [1, 1, 64]
        nc.gpsimd.dma_start(
            q_full.rearrange("p (k d) -> p k d", k=KQ),
            qa_src.broadcast_to((P, KQ, D)),
        )
        ge_sb = cpool.tile([1, 1], f32)
        nc.scalar.dma_start(ge_sb, GE.rearrange("(o d) -> o d", o=1))
        idt = cpool.tile([P, P], bf16)          # identity for PE transpose
        nc.gpsimd.dma_start(idt, IDT)
        # single-group block-diag q / qc at partition base 0 (every KO-group
        # uses the same rhs; only the lhsT w-transpose differs per group)
        wqblk = cpool.tile([KO, KO * D], bf16)
        nc.gpsimd.dma_start(wqblk, WQBLK)
        qcblk = cpool.tile([KO, KO * C], bf16)
        nc.gpsimd.dma_start(qcblk, QCBLK)
        if cfg.get("oa_mult_engine", "gpsimd") == "pe":
            oneblk = cpool.tile([KO, KO * D], bf16)
            nc.gpsimd.dma_start(oneblk, ONEBLK)
        else:
            oneblk = None

        # d_sb holds dist^2 -> dist -> exp(-dist) in place, [P, NE] f32
        d_sb = cpool.tile([P, NE], f32)
        scal = cpool.tile([P, 16], f32)
        gam_b = cpool.tile([P, 1], f32)
        ngam_b = cpool.tile([P, 1], f32)

        # persistent bf16 A-tile cache (the last n_cache tiles incl. tail)
        n_tiles = n_main + (1 if KT else 0)
        n_cache = min(cfg["n_cache"], n_tiles)
        cache = {}
        for t in range(n_tiles - n_cache, n_tiles):
            k = K if t < n_main else KT
            cache[t] = cachepool.tile([P, k * D], bf16, tag=f"c{t}",
                                      name=f"cache{t}")

        for _rep in range(reps):
            _dsdm_one_pass(
                tc, cfg, A_t, M_t, O_t, A_tail, M_tail, O_tail,
                q_full, idt, wqblk, qcblk, oneblk, ge_sb, d_sb, scal, gam_b,
                ngam_b, cache, tpool, opool, mpool, bpool, wspool, psT, psA,
                psM, drampool, cpool,
            )


def _dsdm_one_pass(tc, cfg, A_t, M_t, O_t, A_tail, M_tail, O_tail, q_full,
                   idt, wqblk, qcblk, oneblk, ge_sb, d_sb, scal, gam_b,
                   ngam_b, cache, tpool, opool, mpool, bpool, wspool, psT,
                   psA, psM, drampool, cpool):
    nc = tc.nc
    f32 = mybir.dt.float32
    bf16 = mybir.dt.bfloat16
    K = cfg["k_main"]
    KT = cfg["k_tail"]
    n_main = cfg["n_main"]
    NE = cfg["n_e_cols"]
    KO = cfg["k_outer"]
    n_tiles = n_main + (1 if KT else 0)
    n_cache = min(cfg["n_cache"], n_tiles)
    X = mybir.AxisListType.X
    ALU = mybir.AluOpType
    ACT = mybir.ActivationFunctionType
    sub_dve_frac = cfg.get("sub_dve_frac", 0.4)

    if cfg.get("q_bcast"):
        qf1 = q_full.rearrange("p (k d) -> p k d", k=1)
        qf3 = None
    else:
        qf3 = q_full.rearrange("p (k d) -> p k d", k=K)

    # ---------------- phase A: dist^2 per row ---------------------------
    col = 0
    for t in range(n_tiles):
        k = K if t < n_main else KT
        a_dram = A_t[t] if t < n_main else A_tail
        cached = t in cache
        if cached:
            a = cache[t]
        else:
            a = tpool.tile([P, K * D], bf16, tag="t_in", name="a_in")
        nc.gpsimd.dma_start(a[:, : k * D], a_dram)  # f32 -> bf16 cast load
        if cached:
            tsub = tpool.tile([P, K * D], bf16, tag="t_in", name="t_sub")
        else:
            tsub = a  # uncached tiles are dead after the reduce: sub in place
        a3 = a.rearrange("p (k d) -> p k d", k=K) if k == K else \
            a[:, : k * D].rearrange("p (k d) -> p k d", k=k)
        t3 = tsub.rearrange("p (k d) -> p k d", k=K) if k == K else \
            tsub[:, : k * D].rearrange("p (k d) -> p k d", k=k)
        kv = max(1, min(k, round(sub_dve_frac * k)))
        def _q(lo, hi):
            if qf3 is not None:
                return qf3[:, lo:hi, :]
            return qf1[:, 0:1, :].broadcast_to((P, hi - lo, D))
        nc.vector.tensor_sub(t3[:, :kv, :], a3[:, :kv, :], _q(0, kv))
        if kv < k:
            nc.gpsimd.tensor_sub(t3[:, kv:k, :], a3[:, kv:k, :], _q(kv, k))
        nc.scalar.activation(tsub[:, : k * D], tsub[:, : k * D], ACT.Square)
        if cfg.get("skip_reduce"):
            col += k
            continue
        # tensor_reduce only has a 1x uop on DVE; pre-fold the D axis with
        # tree-halving TT adds (2x mode on dense bf16) before the 1x reduce.
        levels = cfg.get("tree_reduce", 0)
        t4 = tsub[:, : k * D].rearrange("p (k d) -> p k d", k=k)
        dd = D
        for _ in range(levels):
            hd = dd // 2
            nc.vector.tensor_add(t4[:, :, 0:hd], t4[:, :, 0:hd],
                                 t4[:, :, hd:dd])
            dd = hd
        nc.vector.tensor_reduce(d_sb[:, col : col + k], t4[:, :, 0:dd],
                                axis=X, op=ALU.add)
        col += k

    # ---------------- batched d=sqrt(d2), stats, e=exp(-d) in place -----
    nc.scalar.activation(d_sb[:, :NE], d_sb[:, :NE], ACT.Sqrt)  # now dist
    mloc = cpool.tile([P, 1], f32)
    nc.vector.tensor_reduce(mloc, d_sb[:, :NE], axis=X, op=ALU.min)
    sloc = cpool.tile([P, 1], f32)
    nc.scalar.activation(d_sb[:, :NE], d_sb[:, :NE], ACT.Exp, scale=-1.0,
                         accum_out=sloc)  # now exp(-dist)
    # Pad rows were filled host-side with a large constant: d ~ 8e5 so
    # exp(-d) underflows to exactly 0 and the min is unaffected.

    nmloc = cpool.tile([P, 1], f32)
    nc.vector.tensor_scalar_mul(nmloc, mloc, -1.0)
    nm_all = cpool.tile([P, 1], f32)
    s_all = cpool.tile([P, 1], f32)
    nc.gpsimd.partition_all_reduce(nm_all, nmloc, channels=P,
                                   reduce_op=bass_isa.ReduceOp.max)
    nc.gpsimd.partition_all_reduce(s_all, sloc, channels=P,
                                   reduce_op=bass_isa.ReduceOp.add)
    pack = cpool.tile([1, 8], f32)
    nc.vector.tensor_scalar_mul(pack[0:1, 0:1], nm_all[0:1, 0:1], -1.0)
    nc.vector.tensor_copy(pack[0:1, 1:2], s_all[0:1, 0:1])
    nc.vector.memset(pack[0:1, 2:8], 0.0)

    # ---------------- collective: AllGather the 8 (min, sum) pairs ------
    n_cores = cfg["n_cores"]
    if cfg.get("use_collective", True):
        cin = drampool.tile([1, 8], f32)
        cout = drampool.tile([n_cores, 8], f32)
        nc.sync.dma_start(cin, pack)
        nc.gpsimd.collective_compute(
            "AllGather",
            ALU.bypass,
            replica_groups=[list(range(n_cores))],
            ins=[cin[:, :].opt()],
            outs=[cout[:, :].opt()],
        )
        g8 = cpool.tile([n_cores, 8], f32)
        nc.sync.dma_start(g8, cout)

        ng = cpool.tile([n_cores, 1], f32)
        nc.vector.tensor_scalar_mul(ng, g8[:, 0:1], -1.0)
        ng_all = cpool.tile([n_cores, 1], f32)
        z_all = cpool.tile([n_cores, 1], f32)
        nc.gpsimd.partition_all_reduce(ng_all, ng, channels=n_cores,
                                       reduce_op=bass_isa.ReduceOp.max)
        nc.gpsimd.partition_all_reduce(z_all, g8[:, 1:2], channels=n_cores,
                                       reduce_op=bass_isa.ReduceOp.add)
    else:
        ng_all, z_all = nm_all, s_all  # single-core: locals are global

    # ---------------- scalar math on partition 0 ------------------------
    s0 = scal  # [P, 16] scratch; row 0 columns
    mstar = s0[0:1, 0:1]
    nc.vector.tensor_scalar_mul(mstar, ng_all[0:1, 0:1], -1.0)
    zrec = s0[0:1, 1:2]
    nc.vector.reciprocal(zrec, z_all[0:1, 0:1])
    t1 = s0[0:1, 2:3]
    nc.vector.tensor_scalar_mul(t1, mstar, float(EMA_TEMP))
    t2 = s0[0:1, 3:4]
    nc.vector.tensor_scalar_mul(t2, ge_sb, float(1.0 - EMA_TEMP))
    newge = s0[0:1, 4:5]
    nc.vector.tensor_add(newge, t1, t2)
    thr = s0[0:1, 5:6]
    nc.vector.tensor_scalar_mul(thr, newge, float(COEF))
    app = s0[0:1, 6:7]
    nc.vector.tensor_tensor(app, mstar, thr, op=ALU.is_ge)
    keep = s0[0:1, 7:8]
    nc.vector.tensor_scalar(keep, app, -1.0, 1.0, op0=ALU.mult, op1=ALU.add)
    gam1 = s0[0:1, 8:9]
    nc.vector.tensor_mul(gam1, keep, zrec)
    gam = s0[0:1, 9:10]
    nc.vector.tensor_scalar_mul(gam, gam1, float(EMA))
    nc.gpsimd.partition_broadcast(gam_b, gam, channels=P)
    nc.vector.tensor_scalar_mul(ngam_b, gam_b, -1.0)

    if cfg.get("skip_phase_c"):
        return

    # ---------------- phase C: out = a*s + w*q (s = 1 - w, w = gam*e) ---
    # s and w are per-(row) scalars; PE replicates them across columns via
    # outer products (wT@qblk -> w*q, sT@onesblk -> s replicated), ACT
    # bounces PSUM->SBUF as bf16, DVE does two dense bf16 tensor_tensor ops.
    col = 0
    for t in range(n_tiles):
        k = K if t < n_main else KT
        m_dram = M_t[t] if t < n_main else M_tail
        o_dram = O_t[t] if t < n_main else O_tail
        cached = t in cache
        if cached:
            a = cache[t]
        else:
            a = tpool.tile([P, K * D], bf16, tag="t_in", name="a_rd")
            a_dram = A_t[t] if t < n_main else A_tail
            nc.gpsimd.dma_start(a[:, : k * D], a_dram)  # cast re-load
        m = mpool.tile([P, K * C], bf16, tag="m_in", name="m_in")
        nc.gpsimd.dma_start(m[:, : k * C], m_dram)  # cast load

        e_ap = d_sb[:, col : col + k]
        ws = wspool.tile([P, 2 * K], bf16, tag="ws", name="ws")
        w_ap = ws[:, 0:k]
        s_ap = ws[:, K : K + k]
        nc.vector.tensor_scalar_mul(w_ap, e_ap, gam_b[:, 0:1])  # w = gam*e
        nc.vector.tensor_scalar(s_ap, e_ap, ngam_b[:, 0:1], 1.0,
                                op0=ALU.mult, op1=ALU.add)       # s = 1-gam*e

        o = opool.tile([P, K * OD], bf16, tag="o", bufs=2, name="o_tile")
        o3 = o.rearrange("p (k d) -> p k d", k=K)
        a3 = a.rearrange("p (k d) -> p k d", k=K) if k == K else \
            a[:, : k * D].rearrange("p (k d) -> p k d", k=k)
        m3 = m.rearrange("p (k d) -> p k d", k=K) if k == K else \
            m[:, : k * C].rearrange("p (k d) -> p k d", k=k)
        s_bd = s_ap.to_broadcast((P, k, D))
        s_bc = s_ap.to_broadcast((P, k, C))
        # oA = a*s and oM = m*s (broadcast multiply); the PE outer-product
        # corrections are added by DVE below.
        oa_eng = cfg.get("oa_mult_engine", "gpsimd")
        if oa_eng == "gpsimd":
            nc.gpsimd.tensor_tensor(o3[:, :k, 0:D], a3[:, :k, :], s_bd,
                                    op=ALU.mult)
            nc.vector.tensor_tensor(o3[:, :k, D:OD], m3[:, :k, :], s_bc,
                                    op=ALU.mult)
        elif oa_eng == "pe":
            # a*s happens per KO-group below (s replicated on the PE)
            nc.gpsimd.tensor_tensor(o3[:, :k, D:OD], m3[:, :k, :], s_bc,
                                    op=ALU.mult)
        elif oa_eng == "split":
            kh = k // 2
            nc.gpsimd.tensor_tensor(o3[:, :kh, 0:D], a3[:, :kh, :],
                                    s_ap.to_broadcast((P, kh, D)), op=ALU.mult)
            nc.vector.tensor_tensor(o3[:, kh:k, 0:D], a3[:, kh:k, :],
                                    s_ap[:, kh:].to_broadcast((P, k - kh, D)),
                                    op=ALU.mult)
            nc.gpsimd.tensor_tensor(o3[:, :k, D:OD], m3[:, :k, :], s_bc,
                                    op=ALU.mult)
        else:  # dve
            nc.vector.tensor_tensor(o3[:, :k, 0:D], a3[:, :k, :], s_bd,
                                    op=ALU.mult)
            nc.gpsimd.tensor_tensor(o3[:, :k, D:OD], m3[:, :k, :], s_bc,
                                    op=ALU.mult)

        use_pe_srep = oa_eng == "pe"
        for ko in range(0, k, KO):
            ks = min(KO, k - ko)
            wt_ps = psT.tile([KO, P], bf16, tag="wt",
                             bufs=1 if use_pe_srep else 2)
            nc.tensor.transpose(wt_ps[:ks, :], w_ap[:, ko : ko + ks], idt)
            wt_sb = wspool.tile([KO, P], bf16, tag="wt_sb", name="wt_sb")
            nc.scalar.copy(wt_sb[:ks, :], wt_ps[:ks, :])
            pa = psA.tile([P, KO * D], f32, tag="pa", bufs=1)
            for j in range(0, ks * D, 512):
                je = min(j + 512, ks * D)
                nc.tensor.matmul(pa[:, j:je], wt_sb[:ks, :],
                                 wqblk[:ks, j:je], start=True, stop=True)
            pm = psM.tile([P, KO * C], f32, tag="pm",
                          bufs=1 if use_pe_srep else 2)
            nc.tensor.matmul(pm[:, : ks * C], wt_sb[:ks, :],
                             qcblk[:ks, : ks * C], start=True, stop=True)
            pab = bpool.tile([P, KO * D], bf16, tag="pab", name="pab")
            nc.scalar.copy(pab[:, : ks * D], pa[:, : ks * D])
            pab3 = pab.rearrange("p (k d) -> p k d", k=KO)[:, :ks, :]
            oAs = o3[:, ko : ko + ks, 0:D]
            oMs = o3[:, ko : ko + ks, D:OD]
            if use_pe_srep:
                # s replicated across D via PE outer product w^T @ onesblk;
                # the (1 - x) fold happens in the ACT bounce (scale/bias).
                pw = psA.tile([P, KO * D], f32, tag="pw", bufs=1)
                for j in range(0, ks * D, 512):
                    je = min(j + 512, ks * D)
                    nc.tensor.matmul(pw[:, j:je], wt_sb[:ks, :],
                                     oneblk[:ks, j:je], start=True, stop=True)
                psb = bpool.tile([P, KO * D], bf16, tag="psb", name="psb")
                nc.scalar.activation(psb[:, : ks * D], pw[:, : ks * D],
                                     ACT.Copy, bias=1.0, scale=-1.0)
                psb3 = psb.rearrange("p (k d) -> p k d", k=KO)[:, :ks, :]
                a3s = a3[:, ko : ko + ks, :]
                nc.vector.tensor_tensor(oAs, a3s, psb3, op=ALU.mult)  # a*s
            nc.vector.tensor_tensor(oAs, oAs, pab3, op=ALU.add)    # += w*q
            pm3 = pm.rearrange("p (k d) -> p k d", k=KO)[:, :ks, :]
            nc.vector.tensor_tensor(oMs, oMs, pm3, op=ALU.add)     # += w*qc
        nc.gpsimd.dma_start(o_dram, o[:, : k * OD])  # bf16 -> f32 cast write
        col += k


_BUILD_CACHE = {}


def build_nc(cfg):
    key = tuple(sorted(cfg.items()))
    if key in _BUILD_CACHE:
        return _BUILD_CACHE[key]
    nc = bacc.Bacc("TRN2", target_bir_lowering=False, debug=False,
                   num_devices=cfg["n_cores"])
    f32 = mybir.dt.float32
    n_pad = cfg["n_pad"]
    K = cfg["k_main"]
    KO = cfg["k_outer"]
    n_grp = -(-K // KO)
    A = nc.dram_tensor("A", [n_pad, D], f32, kind="ExternalInput").ap()
    M = nc.dram_tensor("M", [n_pad, C], f32, kind="ExternalInput").ap()
    QA = nc.dram_tensor("QA", [D], f32, kind="ExternalInput").ap()
    QC = nc.dram_tensor("QC", [C], f32, kind="ExternalInput").ap()
    GE = nc.dram_tensor("GE", [1], f32, kind="ExternalInput").ap()
    IDT = nc.dram_tensor("IDT", [P, P], f32, kind="ExternalInput").ap()
    WQBLK = nc.dram_tensor("WQBLK", [KO, KO * D], f32,
                           kind="ExternalInput").ap()
    ONEBLK = nc.dram_tensor("ONEBLK", [KO, KO * D], f32,
                            kind="ExternalInput").ap()
    QCBLK = nc.dram_tensor("QCBLK", [KO, KO * C], f32,
                           kind="ExternalInput").ap()
    OUT = nc.dram_tensor("OUT", [n_pad, OD], f32, kind="ExternalOutput").ap()
    with tile.TileContext(nc) as tc:
        dsdm_kernel_body(tc, A, M, QA, QC, GE, IDT, WQBLK, QCBLK, ONEBLK, OUT, cfg)
    nc.compile()
    _BUILD_CACHE[key] = nc
    return nc


PAD_VALUE = 1.0e4  # pad rows -> dist ~8e5 -> exp underflows to 0; min unaffected


def make_aux_inputs(cfg, qa, qc):
    """Host-built constants: identity + single-group block-diag q/qc."""
    k = cfg["k_outer"]
    qblk = np.zeros((k, k * D), np.float32)
    oblk = np.zeros((k, k * D), np.float32)
    qcblk = np.zeros((k, k * C), np.float32)
    for kk in range(k):
        qblk[kk, kk * D : (kk + 1) * D] = qa
        oblk[kk, kk * D : (kk + 1) * D] = 1.0
        qcblk[kk, kk * C : (kk + 1) * C] = qc
    return {
        "IDT": np.eye(P, dtype=np.float32),
        "WQBLK": qblk,
        "ONEBLK": oblk,
        "QCBLK": qcblk,
    }


def _shard_pad(x, n_cores, n_real, n_pad):
    """Split rows across cores, pad each shard to n_pad with PAD_VALUE rows."""
    shards = []
    pad = n_pad - n_real
    for c in range(n_cores):
        s = x[c * n_real : (c + 1) * n_real]
        if pad:
            s = np.concatenate(
                [s, np.full((pad, s.shape[1]), PAD_VALUE, dtype=np.float32)], axis=0
            )
        shards.append(np.ascontiguousarray(s, dtype=np.float32))
    return shards


_WARMED = False


def _warm_devices(n_cores, tries=7, wait=45.0):
    """Touch every core with a trivial op before the real run.

    The axon terminal occasionally reports NRT_EXEC_UNIT_UNRECOVERABLE on the
    first use after another session exited uncleanly, and recovers on its own
    within a couple of minutes — retry cheap ops until the mesh is healthy."""
    global _WARMED
    if _WARMED:
        return
    import time as _time

    import jax
    import jax.numpy as jnp

    last = None
    for t in range(tries):
        try:
            for d in jax.devices()[:n_cores]:
                y = jax.device_put(np.zeros(4, np.float32), d)
                assert float(jnp.sum(y).block_until_ready()) == 0.0
            _WARMED = True
            return
        except Exception as e:  # noqa: BLE001 - retry any backend error
            last = e
            _time.sleep(wait)
    raise RuntimeError(f"NeuronCores unavailable after {tries} tries") from last


def kernel(A, M, query_address, query_content, global_error, _trace=False):
    A = np.asarray(A, dtype=np.float32)
    M = np.asarray(M, dtype=np.float32)
    qa = np.ascontiguousarray(np.asarray(query_address, dtype=np.float32))
    qc = np.ascontiguousarray(np.asarray(query_content, dtype=np.float32))
    ge = np.ascontiguousarray(np.asarray(global_error, dtype=np.float32))

    n_total = A.shape[0]
    n_cores = N_CORES
    assert n_total % n_cores == 0
    n_real = n_total // n_cores
    cfg = make_cfg(n_real)
    nc = build_nc(cfg)
    _warm_devices(n_cores)

    a_sh = _shard_pad(A, n_cores, n_real, cfg["n_pad"])
    m_sh = _shard_pad(M, n_cores, n_real, cfg["n_pad"])
    aux = make_aux_inputs(cfg, qa, qc)
    in_maps = [
        {"A": a_sh[c], "M": m_sh[c], "QA": qa, "QC": qc, "GE": ge, **aux}
        for c in range(n_cores)
    ]
    res = run_bass_kernel_spmd(nc, in_maps, core_ids=list(range(n_cores)),
                               trace=False)
    outs = [res.results[c]["OUT"][:n_real] for c in range(n_cores)]
    full = np.concatenate(outs, axis=0)
    if _trace:
        kernel.last_results = res
    return full


if __name__ == "__main__":
    # smoke test with random data (no reference comparison here)
    rng = np.random.default_rng(0)
    A = rng.standard_normal((N_MEM, D), dtype=np.float32)
    M = rng.standard_normal((N_MEM, C), dtype=np.float32)
    qa = rng.standard_normal(D).astype(np.float32)
    qc = rng.standard_normal(C).astype(np.float32)
    ge = rng.random(1, dtype=np.float32)
    out = kernel(A, M, qa, qc, ge)
    print("out", out.shape, out.dtype, float(np.abs(out).max()))
